# revision 1
# baseline (speedup 1.0000x reference)
"""Trainium2 Bass kernel for nn_AccFlow2FrameEncoder (PointPillars-style encoder).

Math per (batch, cloud):
  voxelize points into 512x512 grid; cluster-mean of xyz per pillar; features
  f = [pts, pts-cm, pts-center] (9); h = relu(BN(f @ W)); scatter-mean h per
  pillar. Output = grid(pc1) - grid(pc0), shape [2, 512, 512, 64] f32.

Sharding: 8 cores = 2 batches x 4 strip-pairs. Each core handles 2 strips of
64 ix-rows (32768 pillars each) x both clouds, so the diff is formed on-core.
Host partitions points by strip, orders them into collision-free "rounds"
(rank within pillar) so every on-device dma_scatter_add call has unique
pillar indices (exact f32 accumulation; duplicate indices in one call race).

Device pipeline per unit (strip x cloud):
  1. dma_scatter_add [x,y,z,w] into SBUF parity tables (unique-idx rounds)
  2. export tables to a component-major DRAM table [4, 32768]
  3. per quarter-strip: load a [128, 8192] replicated SBUF table and ap_gather
     per-point pillar sums (per-Q7-core idx lists); merge quarters by mask
  4. build 9-feature vectors (cluster-mean + center + ones-bias features);
     PE transpose + matmul (9->64); ACT relu scaled by validity w
  5. dma_scatter_add [w, h] (65 f32) into SBUF parity tables (unique rounds)
  6. after both clouds: normalize by counts, subtract, DMA to output grid

(dma_gather/InstDMAGatherAnt is avoided deliberately: it crashes at runtime
under the axon/PJRT path; SBUF-only custom ops work.)
"""
import sys
sys.path.insert(0, '/opt/trn_rl_repo')

import numpy as np

VX = np.float32(0.2)
X_MIN = np.float32(-51.2)
GX = 512
N_STRIPS = 8          # strips of 64 ix rows
SP = 32768            # pillars per strip
R_CAPS = [90, 20, 4, 2, 1, 1, 1, 1]   # per-round capacity in 128-token units
MC = sum(R_CAPS)      # 120 -> M = 15360 tokens per unit
M = MC * 128
N_ROUNDS = len(R_CAPS)


def _chunks(max_cap=24):
    """(offset, size) sub-chunks of the rounds, each <= max_cap 128-token
    units, never crossing a round boundary (keeps per-call indices unique)."""
    out = []
    off = 0
    for cap in R_CAPS:
        o = 0
        while o < cap:
            c = min(max_cap, cap - o)
            out.append((off + o, c))
            o += c
        off += cap
    return out

_nc_cache = {}


def _build_nc():
    if 'nc' in _nc_cache:
        return _nc_cache['nc']
    import concourse.bass as bass
    import concourse.tile as tile
    from concourse import bacc, mybir
    from concourse.bass import AP
    from concourse.masks import make_identity
    from concourse.library_config import mlp, ap_gather as apg_lib

    F32 = mybir.dt.float32
    I16 = mybir.dt.int16
    Alu = mybir.AluOpType
    Act = mybir.ActivationFunctionType

    # raise stale SBUF cap if present
    try:
        import concourse.tile_utils as tile_utils
        if getattr(tile_utils, 'max_sbuf_usage', None):
            tile_utils.max_sbuf_usage = 206 * 1024
    except Exception:
        pass

    nc = bacc.Bacc(None, target_bir_lowering=False, debug=False)

    pts_d = [nc.dram_tensor(f"pts{u}", [M, 4], F32, kind="ExternalInput") for u in range(4)]
    wc_d = [nc.dram_tensor(f"wc{u}", [9, 64], F32, kind="ExternalInput") for u in range(4)]
    scr_d = [nc.dram_tensor(f"scr{u}", [5 * M], I16, kind="Internal") for u in range(4)]
    t1t_d = [nc.dram_tensor(f"t1t{u}", [4, SP], F32, kind="Internal") for u in range(2)]
    scr2_d = [nc.dram_tensor(f"scr2_{u}", [4, 4, M], F32, kind="Internal") for u in range(2)]
    out_d = nc.dram_tensor("out", [2 * SP, 64], F32, kind="ExternalOutput")

    F23 = float(2 ** 23)

    with tile.TileContext(nc) as tc:
        with tc.tile_pool(name="sb", bufs=1) as pool, \
             tc.tile_pool(name="sb2", bufs=2) as pool2, \
             tc.tile_pool(name="ps", bufs=2, space="PSUM") as psum, \
             tc.tile_pool(name="psq", bufs=4, space="PSUM") as psumq:

            nc.gpsimd.load_library(mlp)
            ident = pool.tile([128, 128], F32, tag="ident")
            make_identity(nc, ident[:])

            def floor_pos(dst, src, tmp):
                # dst = floor(src) for src >= 0 (RNE-to-int via +2^23 then correct)
                nc.vector.tensor_scalar_add(tmp[:], src[:], F23)
                nc.vector.tensor_scalar_add(dst[:], tmp[:], -F23)
                nc.vector.tensor_tensor(out=tmp[:], in0=dst[:], in1=src[:], op=Alu.is_gt)
                nc.vector.tensor_tensor(out=dst[:], in0=dst[:], in1=tmp[:], op=Alu.subtract)

            t2tabs = {}
            import os as _os
            _skip = set(_os.environ.get("KERNEL_SKIP", "").split(","))
            _reps = int(_os.environ.get("KERNEL_REPS", "1"))
            for _rep in range(_reps):
              for u in range(4):
                strip_l, cloud = u // 2, u % 2

                pts_t = pool.tile([128, MC, 4], F32, tag="pts", name=f"pts_t{u}")
                nc.sync.dma_start(pts_t[:], pts_d[u].ap().rearrange("(j p) c -> p j c", p=128))
                wc_t = pool2.tile([9, 64], F32, tag="wc")
                nc.sync.dma_start(wc_t[:], wc_d[u].ap())

                # ---- per-point voxel coords in data layout [128, MC]
                def T(tag):
                    return pool.tile([128, MC], F32, tag=tag, name=f"{tag}_u{u}")
                ux, tmp = T("ux"), T("tmp")
                ixf, iyf = T("ixf"), T("iyf")
                nc.vector.tensor_scalar_mul(ux[:], pts_t[:, :, 0], 5.0)
                floor_pos(ixf, ux, tmp)
                nc.vector.tensor_scalar_max(ixf[:], ixf[:], 0.0)
                nc.vector.tensor_scalar_min(ixf[:], ixf[:], 63.0)
                nc.vector.tensor_scalar_mul(ux[:], pts_t[:, :, 1], 5.0)
                floor_pos(iyf, ux, tmp)
                nc.vector.tensor_scalar_max(iyf[:], iyf[:], 0.0)
                nc.vector.tensor_scalar_min(iyf[:], iyf[:], 511.0)
                vidf = T("vidf")
                nc.vector.tensor_scalar_mul(vidf[:], ixf[:], 512.0)
                nc.vector.tensor_tensor(out=vidf[:], in0=vidf[:], in1=iyf[:], op=Alu.add)
                # quarter id (vid >> 13) for 4-way gather merge
                mQ = T("mQ")
                nc.vector.tensor_scalar_mul(mQ[:], vidf[:], float(2 ** -13))
                floor_pos(ux, mQ, tmp)
                nc.vector.tensor_copy(mQ[:], ux[:])

                # ---- idx values f32->i16, roundtrip to wrap-16 / per-core wrap
                i16t = pool.tile([128, MC], I16, tag="i16t")
                vH = T("vH")
                nc.vector.tensor_copy(i16t[:], vidf[:])
                dst = bass.AP(scr_d[u], 0, [[1, 128], [128, MC]])
                nc.sync.dma_start(dst, i16t[:])
                for q in range(4):
                    nc.vector.tensor_scalar_add(vH[:], vidf[:], -8192.0 * q)
                    nc.vector.tensor_scalar_max(vH[:], vH[:], 0.0)
                    nc.vector.tensor_scalar_min(vH[:], vH[:], 8191.0)
                    nc.vector.tensor_copy(i16t[:], vH[:])
                    dst = bass.AP(scr_d[u], (1 + q) * M, [[1, 128], [128, MC]])
                    nc.sync.dma_start(dst, i16t[:])

                idxv = pool.tile([128, MC * 8], I16, tag="gbuf")
                if "idx" in _skip:
                    nc.vector.memset(idxv[:], 0)
                for k in (() if "idx" in _skip else range(8)):
                    srcap = bass.AP(scr_d[u], 0, [[1, 16], [16, MC * 8]])
                    nc.sync.dma_start(idxv[16 * k:16 * k + 16, :], srcap)
                idxQ = [pool.tile([128, MC], I16, tag=f"idxQ{q}",
                                  name=f"idxQ{q}_u{u}") for q in range(4)]
                for q in range(4):
                    if "idx" in _skip:
                        nc.vector.memset(idxQ[q][:], 0)
                        continue
                    for c in range(8):
                        srcap = bass.AP(scr_d[u], (1 + q) * M + 1920 * c,
                                        [[1, 16], [16, MC]])
                        nc.sync.dma_start(idxQ[q][16 * c:16 * c + 16, :], srcap)

                # ---- T1 scatter rounds (xyzw, elem 4)
                t1A = pool.tile([128, 128, 4], F32, tag="t1A")
                t1B = pool.tile([128, 128, 4], F32, tag="t1B")
                nc.vector.memset(t1A[:], 0.0)
                nc.vector.memset(t1B[:], 0.0)
                for off, cap in (() if "t1" in _skip else _chunks()):
                    n = cap * 128
                    nc.gpsimd.dma_scatter_add(
                        t1A[:], pts_t[:, off:off + cap, :],
                        idxv[:, off * 8:(off + cap) * 8],
                        n, n, 4,
                        sbuf_tokens_per_rank=128, parity_reg=0, out_ap_other=t1B[:])

                # ---- export component-major table [4, 32768]
                t1t = t1t_d[cloud]
                for par, tbl in ((0, t1A), (1, t1B)):
                    for r in range(4):
                        dst = bass.AP(t1t, 32768 * r + 128 * par,
                                      [[1, 128], [256, 128]])
                        nc.sync.dma_start(dst, tbl[:, :, r])

                # ---- ap_gather per quarter-strip (SBUF table, per-core idx)
                nc.gpsimd.load_library(apg_lib)
                tab = pool.tile([128, 8192], F32, tag="big", name=f"tab{u}")
                if _os.environ.get("KERNEL_SIM"):
                    # only CoreSim's uninit-memory check needs this; on HW the
                    # pad rows are never consumed
                    nc.vector.memset(tab[:], 0.0)
                for q in (() if "cm" in _skip else range(4)):
                    for c in range(8):
                        srcap = bass.AP(t1t, q * 8192,
                                        [[32768, 4], [1, 8192]])
                        nc.sync.dma_start(tab[16 * c:16 * c + 4, :], srcap)
                    gb = pool.tile([128, MC * 16], F32, tag="gout",
                                   name=f"gout{u}_{q}")
                    if "gath" not in _skip:
                        nc.gpsimd.ap_gather(gb[:], tab[:], idxQ[q][:],
                                            128, 8192, 1, MC * 16)
                    for c in range(8):
                        dst = bass.AP(scr2_d[cloud], q * 4 * M + 1920 * c,
                                      [[M, 4], [1, 1920]])
                        nc.sync.dma_start(dst, gb[16 * c:16 * c + 4, :])
                nc.gpsimd.load_library(mlp)
                sel = pool.tile([128, MC, 4], F32, tag="sel")
                selL = pool.tile([128, MC, 4], F32, tag="selL")
                tmp4 = pool.tile([128, MC, 4], F32, tag="tmp4")
                mb = T("mb")
                nc.vector.memset(sel[:], 0.0)
                if "cm" in _skip:
                    nc.vector.memset(selL[:], 0.0)
                for q in (() if "cm" in _skip else range(4)):
                    for r in range(4):
                        srcap = bass.AP(scr2_d[cloud], q * 4 * M + r * M,
                                        [[1, 128], [128, MC]])
                        nc.sync.dma_start(selL[:, :, r], srcap)
                    nc.vector.tensor_scalar(mb[:], mQ[:], float(q), None, op0=Alu.is_equal)
                    nc.vector.tensor_tensor(out=tmp4[:], in0=selL[:],
                                            in1=mb[:].to_broadcast([128, MC, 4]),
                                            op=Alu.mult)
                    nc.vector.tensor_tensor(out=sel[:], in0=sel[:], in1=tmp4[:], op=Alu.add)

                rw = T("rw")
                nc.vector.tensor_scalar_max(tmp[:], sel[:, :, 3], 1.0)
                nc.vector.reciprocal(rw[:], tmp[:])

                # ---- feature tensor fc [128, MC, 16] (cols 9:16 zero pad)
                fc = pool.tile([128, MC, 16], F32, tag="fc")
                nc.vector.memset(fc[:], 0.0)
                nc.vector.memset(fc[:, :, 8], 1.0)
                nc.vector.tensor_copy(fc[:, :, 0:3], pts_t[:, :, 0:3])
                tmp3 = pool.tile([128, MC, 3], F32, tag="tmp3")
                nc.vector.tensor_tensor(out=tmp3[:], in0=sel[:, :, 0:3],
                                        in1=rw[:].to_broadcast([128, MC, 3]), op=Alu.mult)
                nc.vector.tensor_tensor(out=fc[:, :, 3:6], in0=pts_t[:, :, 0:3],
                                        in1=tmp3[:], op=Alu.subtract)
                nc.vector.scalar_tensor_tensor(
                    out=fc[:, :, 6], in0=ixf[:], scalar=-float(VX), in1=pts_t[:, :, 0],
                    op0=Alu.mult, op1=Alu.add)
                nc.vector.tensor_scalar_add(fc[:, :, 6], fc[:, :, 6], -float(VX) / 2)
                nc.vector.scalar_tensor_tensor(
                    out=fc[:, :, 7], in0=iyf[:], scalar=-float(VX), in1=pts_t[:, :, 1],
                    op0=Alu.mult, op1=Alu.add)
                nc.vector.tensor_scalar_add(fc[:, :, 7], fc[:, :, 7], -float(VX) / 2)

                # ---- PE: transpose + matmul, ACT relu
                h_t = pool.tile([128, MC, 65], F32, tag="big")
                nc.vector.tensor_copy(h_t[:, :, 0], pts_t[:, :, 3])
                for ch in (() if "pe" in _skip else range(MC)):
                    pt = psum.tile([16, 128], F32, tag="pt")
                    nc.tensor.transpose(
                        out=pt[:],
                        in_=fc[:, ch, :],
                        identity=ident[:])
                    fT = pool2.tile([16, 128], F32, tag="fT")
                    nc.vector.tensor_copy(fT[:], pt[:])
                    qp = psumq.tile([128, 64], F32, tag="q")
                    nc.tensor.matmul(out=qp[:], lhsT=fT[0:9, :],
                                     rhs=wc_t[:], start=True, stop=True)
                    nc.scalar.activation(h_t[:, ch, 1:65], qp[:], Act.Relu,
                                         scale=pts_t[:, ch, 3:4])

                # ---- T2 scatter rounds (w+h, elem 65)
                t2A = pool.tile([128, 128, 65], F32, tag=f"t2A{cloud}")
                t2B = pool.tile([128, 128, 65], F32, tag=f"t2B{cloud}")
                nc.vector.memset(t2A[:], 0.0)
                nc.vector.memset(t2B[:], 0.0)
                for off, cap in (() if "t2" in _skip else _chunks()):
                    n = cap * 128
                    nc.gpsimd.dma_scatter_add(
                        t2A[:], h_t[:, off:off + cap, :],
                        idxv[:, off * 8:(off + cap) * 8],
                        n, n, 65,
                        sbuf_tokens_per_rank=128, parity_reg=0, out_ap_other=t2B[:])
                t2tabs[(cloud, 0)] = t2A
                t2tabs[(cloud, 1)] = t2B

                # ---- P5: normalize, diff, export (after cloud 1)
                if cloud == 1:
                    for par in range(2):
                        s0 = t2tabs[(0, par)]
                        s1 = t2tabs[(1, par)]
                        cn = pool.tile([128, 128], F32, tag="cn", name=f"cn_{u}_{par}")
                        r0 = pool.tile([128, 128], F32, tag="r0", name=f"r0_{u}_{par}")
                        r1 = pool.tile([128, 128], F32, tag="r1", name=f"r1_{u}_{par}")
                        nc.vector.tensor_scalar_max(cn[:], s0[:, :, 0], 1.0)
                        nc.vector.reciprocal(r0[:], cn[:])
                        nc.vector.tensor_scalar_max(cn[:], s1[:, :, 0], 1.0)
                        nc.vector.reciprocal(r1[:], cn[:])
                        nc.vector.tensor_tensor(
                            out=s1[:, :, 1:65], in0=s1[:, :, 1:65],
                            in1=r1[:].to_broadcast([128, 128, 64]), op=Alu.mult)
                        nc.gpsimd.tensor_tensor(
                            out=s0[:, :, 1:65], in0=s0[:, :, 1:65],
                            in1=r0[:].to_broadcast([128, 128, 64]), op=Alu.mult)
                        nc.vector.tensor_tensor(
                            out=s1[:, :, 1:65], in0=s1[:, :, 1:65],
                            in1=s0[:, :, 1:65], op=Alu.subtract)
                        dst = bass.AP(out_d, (strip_l * SP + 128 * par) * 64,
                                      [[64, 128], [16384, 128], [1, 64]])
                        nc.sync.dma_start(dst, s1[:, :, 1:65])

    nc.compile()
    _nc_cache['nc'] = nc
    return nc


# ---------------- host-side preparation ----------------

def _voxel_host(pts):
    # identical f32 ops to the reference
    x = pts[:, 0].astype(np.float32)
    y = pts[:, 1].astype(np.float32)
    ux = (x - X_MIN) / VX
    uy = (y - X_MIN) / VX
    ix = np.clip(np.floor(ux), 0, 511).astype(np.int32)
    iy = np.clip(np.floor(uy), 0, 511).astype(np.int32)
    return ix, iy


def _prep_unit(pts, strip, vox=None):
    """pts: [N, 3] points of one cloud. Returns [M, 4] f32 (xr, yr, z, w) in
    round-sorted order for points whose global ix falls in this strip."""
    ix, iy = vox if vox is not None else _voxel_host(pts)
    m = (ix >> 6) == strip
    p = pts[m].astype(np.float32)
    ixs = ix[m] - 64 * strip
    iys = iy[m]
    n = p.shape[0]
    assert n <= M, (n, M)

    xsh = np.float32(X_MIN + strip * np.float64(64) * np.float64(VX))
    xr = (p[:, 0] - xsh).astype(np.float32)
    yr = (p[:, 1] - X_MIN).astype(np.float32)
    z = p[:, 2].astype(np.float32)

    # nudge until device arithmetic (u = rel / VX, floor, clip) agrees
    for coord, tgt, hi in ((0, ixs, 63), (1, iys, 511)):
        arr = xr if coord == 0 else yr
        for it in range(6):
            u = (arr * np.float32(5.0)).astype(np.float32)
            dev = np.clip(np.floor(u), 0, hi).astype(np.int32)
            bad = dev != tgt
            if not bad.any():
                break
            eps = np.float32(1e-3 * (it + 1))
            lo_v = ((tgt[bad].astype(np.float32) + eps) * VX).astype(np.float32)
            hi_v = ((tgt[bad].astype(np.float32) + 1 - eps) * VX).astype(np.float32)
            arr[bad] = np.clip(arr[bad], lo_v, hi_v)
        else:
            raise AssertionError("voxel nudge failed")

    vid = ixs * 512 + iys
    # rank within pillar
    order = np.argsort(vid, kind="stable")
    sv = vid[order]
    first = np.ones(n, bool)
    first[1:] = sv[1:] != sv[:-1]
    seg_start = np.maximum.accumulate(np.where(first, np.arange(n), 0))
    rank_sorted = np.arange(n) - seg_start
    rank = np.empty(n, np.int64)
    rank[order] = rank_sorted

    # empty pillar for padding
    used = np.zeros(SP, bool)
    used[vid] = True
    E = int(np.argmin(used))
    assert not used[E]
    ex = np.float32((E // 512 + 0.5) * 0.2)
    ey = np.float32((E % 512 + 0.5) * 0.2)

    out = np.empty((M, 4), np.float32)
    out[:, 0] = ex
    out[:, 1] = ey
    out[:, 2] = 0.0
    out[:, 3] = 0.0
    off = 0
    for r, cap in enumerate(R_CAPS):
        s = np.where(rank == r)[0] if r < N_ROUNDS - 1 else np.where(rank >= r)[0]
        if r == N_ROUNDS - 1:
            assert (rank[s] == r).all(), "rank overflow beyond round capacity"
        assert len(s) <= cap * 128, (r, len(s))
        out[off:off + len(s), 0] = xr[s]
        out[off:off + len(s), 1] = yr[s]
        out[off:off + len(s), 2] = z[s]
        out[off:off + len(s), 3] = 1.0
        off += cap * 128
    return out


def kernel(pc0s, pc1s, W, bn_gamma, bn_beta, bn_mean, bn_var):
    from concourse.bass_utils import run_bass_kernel_spmd

    pc0s = np.asarray(pc0s, np.float32)
    pc1s = np.asarray(pc1s, np.float32)
    W = np.asarray(W, np.float32)
    scale = (np.asarray(bn_gamma, np.float32)
             / np.sqrt(np.asarray(bn_var, np.float32) + np.float32(1e-3)))
    W_eff = W * scale[None, :]
    bias = np.asarray(bn_beta, np.float32) - np.asarray(bn_mean, np.float32) * scale
    W0, W1, W2 = W_eff[0:3], W_eff[3:6], W_eff[6:9]

    def wc_for(strip):
        Sx = np.float32(X_MIN + strip * 12.8)
        Sy = X_MIN
        wc = np.zeros((9, 64), np.float32)
        wc[0] = W0[0]
        wc[1] = W0[1]
        wc[2] = W0[2] + W2[2]
        wc[3:6] = W1
        wc[6] = W2[0]
        wc[7] = W2[1]
        wc[8] = bias + Sx * W0[0] + Sy * W0[1]
        return wc

    clouds = [pc0s, pc1s]
    vox = {(c, b): _voxel_host(clouds[c][b]) for c in range(2) for b in range(2)}
    wcs = [wc_for(s) for s in range(N_STRIPS)]
    in_maps = []
    for core in range(8):
        b, sp = core // 4, core % 4
        ins = {}
        for u in range(4):
            strip = 2 * sp + u // 2
            cloud = u % 2
            ins[f"pts{u}"] = _prep_unit(clouds[cloud][b], strip, vox[(cloud, b)])
            ins[f"wc{u}"] = wcs[strip]
        in_maps.append(ins)

    nc = _build_nc()
    import time as _time
    _t0 = _time.time()
    res = run_bass_kernel_spmd(nc, in_maps, list(range(8)))
    kernel.last_device_wall = _time.time() - _t0

    out = np.zeros((2, GX * GX, 64), np.float32)
    for core in range(8):
        b, sp = core // 4, core % 4
        o = res.results[core]["out"]        # [2*SP, 64]
        for sl in range(2):
            strip = 2 * sp + sl
            out[b, strip * SP:(strip + 1) * SP] = o[sl * SP:(sl + 1) * SP]
    return out.reshape(2, GX, GX, 64)


if __name__ == "__main__":
    # smoke test against reference (reference on CPU backend)
    sys.path.insert(0, '/root/problem')
    import jax
    import reference
    cpu = jax.devices("cpu")[0]
    with jax.default_device(cpu):
        inputs = {k: np.asarray(v) for k, v in reference.setup_inputs().items()}
        exp = np.asarray(reference.reference(**inputs))
    got = kernel(**inputs)
    err = np.abs(got - exp)
    den = np.abs(exp).max()
    print("max abs err:", err.max(), " absmax:", den, " rel:", err.max() / den)



# revision 6
# speedup vs baseline: 1.5289x; 1.5289x over previous
"""Trainium2 Bass kernel for nn_AccFlow2FrameEncoder (PointPillars-style encoder).

Math per (batch, cloud):
  voxelize points into 512x512 grid; cluster-mean of xyz per pillar; features
  f = [pts, pts-cm, pts-center] (9); h = relu(BN(f @ W)); scatter-mean h per
  pillar. Output = grid(pc1) - grid(pc0), shape [2, 512, 512, 64] f32.

Sharding: 8 cores = 2 batches x 4 strip-pairs. Each core handles 2 strips of
64 ix-rows (32768 pillars each) x both clouds, so the diff is formed on-core.
Host partitions points by strip, orders them into collision-free "rounds"
(rank within pillar) so every on-device dma_scatter_add call has unique
pillar indices (exact f32 accumulation; duplicate indices in one call race).
Host also precomputes the wrapped i16 index lists (scatter + per-quarter
gather) and per-token aux data (pillar-center coords, quarter id), loaded as
contiguous DMA inputs -- no on-device index arithmetic or DRAM round trips.

Device pipeline per unit (strip x cloud):
  1. dma_scatter_add [x,y,z,w] into SBUF parity tables (unique-idx rounds)
  2. PE-transpose the parity tables, DMA to a component-major DRAM table
     [4, 32768] with 512B-contiguous descriptors
  3. per quarter-strip: load a [128, 8192] replicated SBUF table and ap_gather
     per-point pillar sums (per-Q7-core idx lists); merge quarters by mask
  4. build 9-feature vectors (cluster-mean + center + ones-bias features);
     PE transpose + matmul (9->64); ACT relu scaled by validity w
  5. dma_scatter_add [w, h] (65 f32) into SBUF parity tables (unique rounds)
  6. after both clouds: normalize by counts, subtract, DMA to output grid

(dma_gather/InstDMAGatherAnt is avoided deliberately: it crashes at runtime
under the axon/PJRT path; SBUF-only custom ops work.)
"""
import sys
sys.path.insert(0, '/opt/trn_rl_repo')

import numpy as np

VX = np.float32(0.2)
X_MIN = np.float32(-51.2)
GX = 512
N_STRIPS = 8          # strips of 64 ix rows
SP = 32768            # pillars per strip
R_CAPS = [90, 20, 4, 2, 1, 1, 1, 1]   # per-round capacity in 128-token units
MC = sum(R_CAPS)      # 120 -> M = 15360 tokens per unit
M = MC * 128
N_ROUNDS = len(R_CAPS)


def _chunks(max_cap=24):
    """(offset, size) sub-chunks of the rounds, each <= max_cap 128-token
    units, never crossing a round boundary (keeps per-call indices unique)."""
    out = []
    off = 0
    for cap in R_CAPS:
        o = 0
        while o < cap:
            c = min(max_cap, cap - o)
            out.append((off + o, c))
            o += c
        off += cap
    return out

_nc_cache = {}


def _build_nc():
    if 'nc' in _nc_cache:
        return _nc_cache['nc']
    import concourse.bass as bass
    import concourse.tile as tile
    from concourse import bacc, mybir
    from concourse.bass import AP
    from concourse.masks import make_identity
    from concourse.library_config import mlp, ap_gather as apg_lib

    F32 = mybir.dt.float32
    I16 = mybir.dt.int16
    Alu = mybir.AluOpType
    Act = mybir.ActivationFunctionType

    # raise stale SBUF cap if present
    try:
        import concourse.tile_utils as tile_utils
        if getattr(tile_utils, 'max_sbuf_usage', None):
            tile_utils.max_sbuf_usage = 206 * 1024
    except Exception:
        pass

    nc = bacc.Bacc(None, target_bir_lowering=False, debug=False)

    pts_d = [nc.dram_tensor(f"pts{u}", [128, MC * 4], F32, kind="ExternalInput") for u in range(4)]
    aux_d = [nc.dram_tensor(f"aux{u}", [128, MC * 4], F32, kind="ExternalInput") for u in range(4)]
    idxv_d = [nc.dram_tensor(f"idxv{u}", [128, MC * 8], I16, kind="ExternalInput") for u in range(4)]
    idxq_d = [nc.dram_tensor(f"idxq{u}", [128, MC * 4], I16, kind="ExternalInput") for u in range(4)]
    wc_d = [nc.dram_tensor(f"wc{u}", [9, 64], F32, kind="ExternalInput") for u in range(4)]
    t1t_d = [nc.dram_tensor(f"t1t{u}", [4, SP], F32, kind="Internal") for u in range(2)]
    scr2_d = [nc.dram_tensor(f"scr2_{u}", [4, 4, M], F32, kind="Internal") for u in range(2)]
    out_d = nc.dram_tensor("out", [2 * SP, 64], F32, kind="ExternalOutput")

    with tile.TileContext(nc) as tc:
        with tc.tile_pool(name="sb", bufs=1) as pool, \
             tc.tile_pool(name="sb2", bufs=2) as pool2, \
             tc.tile_pool(name="ps", bufs=2, space="PSUM") as psum, \
             tc.tile_pool(name="psq", bufs=4, space="PSUM") as psumq:

            nc.gpsimd.load_library(mlp)
            ident = pool.tile([128, 128], F32, tag="ident")
            make_identity(nc, ident[:])

            t2tabs = {}
            import os as _os
            _skip = set(_os.environ.get("KERNEL_SKIP", "").split(","))
            _reps = int(_os.environ.get("KERNEL_REPS", "1"))
            for _rep in range(_reps):
              for u in range(4):
                strip_l, cloud = u // 2, u % 2

                pts_t = pool2.tile([128, MC, 4], F32, tag="pts")
                nc.sync.dma_start(pts_t[:], pts_d[u].ap())
                aux_t = pool2.tile([128, MC, 4], F32, tag="aux")
                nc.sync.dma_start(aux_t[:], aux_d[u].ap())
                idxv = pool2.tile([128, MC * 8], I16, tag="idxv")
                nc.sync.dma_start(idxv[:], idxv_d[u].ap())
                idxQs = pool2.tile([128, MC * 4], I16, tag="idxq")
                nc.sync.dma_start(idxQs[:], idxq_d[u].ap())
                wc_t = pool2.tile([9, 64], F32, tag="wc")
                nc.sync.dma_start(wc_t[:], wc_d[u].ap())

                def T(tag):
                    return pool.tile([128, MC], F32, tag=tag, name=f"{tag}_u{u}")
                tmp = T("tmp")

                # ---- T1 scatter rounds (xyzw, elem 4)
                t1A = pool.tile([128, 128, 4], F32, tag="t1A")
                t1B = pool.tile([128, 128, 4], F32, tag="t1B")
                nc.vector.memset(t1A[:], 0.0)
                nc.vector.memset(t1B[:], 0.0)
                for off, cap in (() if "t1" in _skip else _chunks()):
                    n = cap * 128
                    nc.gpsimd.dma_scatter_add(
                        t1A[:], pts_t[:, off:off + cap, :],
                        idxv[:, off * 8:(off + cap) * 8],
                        n, n, 4,
                        sbuf_tokens_per_rank=128, parity_reg=0, out_ap_other=t1B[:])

                # ---- export component-major table [4, 32768] via PE transpose
                # (512B-contiguous descriptors instead of 4B)
                t1t = t1t_d[cloud]
                for par, tbl in ((0, t1A), (1, t1B)):
                    ptT = psum.tile([128, 4, 128], F32, tag="t1T")
                    for r in range(4):
                        nc.tensor.transpose(
                            out=ptT[:, r, :], in_=tbl[:, :, r], identity=ident[:])
                    t1sb = pool.tile([128, 4, 128], F32, tag="t1Tsb")
                    nc.vector.tensor_copy(t1sb[:], ptT[:])
                    dst = bass.AP(t1t, 128 * par,
                                  [[256, 128], [32768, 4], [1, 128]])
                    nc.sync.dma_start(dst, t1sb[:])

                # ---- ap_gather per quarter-strip (SBUF table, per-core idx);
                # gather output is per-core (token i of core c at row 16c+r,
                # col i): PE-transpose [16,128] blocks into per-token layout
                # in PSUM, then quarter-select straight into sel.
                nc.gpsimd.load_library(apg_lib)
                tab = pool.tile([128, 8192], F32, tag="big", name=f"tab{u}")
                if _os.environ.get("KERNEL_SIM"):
                    # only CoreSim's uninit-memory check needs this; on HW the
                    # pad rows are never consumed
                    nc.vector.memset(tab[:], 0.0)
                sel = pool.tile([128, MC, 4], F32, tag="sel")
                tmp4 = pool.tile([128, MC, 4], F32, tag="tmp4")
                mb = T("mb")
                if _os.environ.get("KERNEL_SIM") or "cm" in _skip:
                    nc.vector.memset(sel[:], 0.0)
                for q in (() if "cm" in _skip else range(4)):
                    for c in range(8):
                        srcap = bass.AP(t1t, q * 8192,
                                        [[32768, 4], [1, 8192]])
                        nc.sync.dma_start(tab[16 * c:16 * c + 4, :], srcap)
                    gb = pool.tile([128, MC * 16], F32, tag="gout",
                                   name=f"gout{u}_{q}")
                    if "gath" not in _skip:
                        nc.gpsimd.ap_gather(gb[:], tab[:],
                                            idxQs[:, q * MC:(q + 1) * MC],
                                            128, 8192, 1, MC * 16)
                    nc.vector.tensor_scalar(mb[:], aux_t[:, :, 2], float(q),
                                            None, op0=Alu.is_equal)
                    for bk in range(4):
                        pq = psum.tile([128, 30, 16], F32, tag=f"pq{bk}")
                        for jj in range(30):
                            j = 30 * bk + jj
                            c, w = j // 15, j % 15
                            nc.tensor.transpose(
                                out=pq[:, jj, :],
                                in_=gb[16 * c:16 * c + 16, 128 * w:128 * w + 128],
                                identity=ident[:])
                        nc.vector.copy_predicated(
                            sel[:, 30 * bk:30 * bk + 30, :],
                            mb[:, 30 * bk:30 * bk + 30].to_broadcast([128, 30, 4]),
                            pq[:, :, 0:4])
                nc.gpsimd.load_library(mlp)

                rw = T("rw")
                nc.vector.tensor_scalar_max(tmp[:], sel[:, :, 3], 1.0)
                nc.vector.reciprocal(rw[:], tmp[:])

                # ---- feature tensor fc [128, MC, 16] (cols 9:16 zero pad)
                fc = pool.tile([128, MC, 16], F32, tag="fc")
                nc.vector.memset(fc[:], 0.0)
                nc.vector.memset(fc[:, :, 8], 1.0)
                nc.vector.tensor_copy(fc[:, :, 0:3], pts_t[:, :, 0:3])
                tmp3 = tmp4[:, :, 0:3]
                nc.vector.tensor_tensor(out=tmp3, in0=sel[:, :, 0:3],
                                        in1=rw[:].to_broadcast([128, MC, 3]), op=Alu.mult)
                nc.vector.tensor_tensor(out=fc[:, :, 3:6], in0=pts_t[:, :, 0:3],
                                        in1=tmp3, op=Alu.subtract)
                nc.vector.tensor_tensor(out=fc[:, :, 6], in0=pts_t[:, :, 0],
                                        in1=aux_t[:, :, 0], op=Alu.subtract)
                nc.vector.tensor_tensor(out=fc[:, :, 7], in0=pts_t[:, :, 1],
                                        in1=aux_t[:, :, 1], op=Alu.subtract)

                # ---- PE: transpose + matmul, ACT relu
                h_t = pool.tile([128, MC, 65], F32, tag="big")
                nc.vector.tensor_copy(h_t[:, :, 0], pts_t[:, :, 3])
                for ch in (() if "pe" in _skip else range(MC)):
                    pt = psum.tile([16, 128], F32, tag="pt")
                    nc.tensor.transpose(
                        out=pt[:],
                        in_=fc[:, ch, :],
                        identity=ident[:])
                    fT = pool2.tile([16, 128], F32, tag="fT")
                    nc.vector.tensor_copy(fT[:], pt[:])
                    qp = psumq.tile([128, 64], F32, tag="q")
                    nc.tensor.matmul(out=qp[:], lhsT=fT[0:9, :],
                                     rhs=wc_t[:], start=True, stop=True)
                    nc.scalar.activation(h_t[:, ch, 1:65], qp[:], Act.Relu,
                                         scale=pts_t[:, ch, 3:4])

                # ---- T2 scatter rounds (w+h, elem 65)
                t2A = pool.tile([128, 128, 65], F32, tag=f"t2A{cloud}")
                t2B = pool.tile([128, 128, 65], F32, tag=f"t2B{cloud}")
                nc.vector.memset(t2A[:], 0.0)
                nc.vector.memset(t2B[:], 0.0)
                for off, cap in (() if "t2" in _skip else _chunks()):
                    n = cap * 128
                    nc.gpsimd.dma_scatter_add(
                        t2A[:], h_t[:, off:off + cap, :],
                        idxv[:, off * 8:(off + cap) * 8],
                        n, n, 65,
                        sbuf_tokens_per_rank=128, parity_reg=0, out_ap_other=t2B[:])
                t2tabs[(cloud, 0)] = t2A
                t2tabs[(cloud, 1)] = t2B

                # ---- P5: normalize, diff, export (after cloud 1)
                if cloud == 1:
                    for par in range(2):
                        s0 = t2tabs[(0, par)]
                        s1 = t2tabs[(1, par)]
                        cn = pool.tile([128, 128], F32, tag="cn", name=f"cn_{u}_{par}")
                        r0 = pool.tile([128, 128], F32, tag="r0", name=f"r0_{u}_{par}")
                        r1 = pool.tile([128, 128], F32, tag="r1", name=f"r1_{u}_{par}")
                        nc.vector.tensor_scalar_max(cn[:], s0[:, :, 0], 1.0)
                        nc.vector.reciprocal(r0[:], cn[:])
                        nc.vector.tensor_scalar_max(cn[:], s1[:, :, 0], 1.0)
                        nc.vector.reciprocal(r1[:], cn[:])
                        nc.vector.tensor_tensor(
                            out=s1[:, :, 1:65], in0=s1[:, :, 1:65],
                            in1=r1[:].to_broadcast([128, 128, 64]), op=Alu.mult)
                        nc.gpsimd.tensor_tensor(
                            out=s0[:, :, 1:65], in0=s0[:, :, 1:65],
                            in1=r0[:].to_broadcast([128, 128, 64]), op=Alu.mult)
                        nc.vector.tensor_tensor(
                            out=s1[:, :, 1:65], in0=s1[:, :, 1:65],
                            in1=s0[:, :, 1:65], op=Alu.subtract)
                        dst = bass.AP(out_d, (strip_l * SP + 128 * par) * 64,
                                      [[64, 128], [16384, 128], [1, 64]])
                        nc.sync.dma_start(dst, s1[:, :, 1:65])

    nc.compile()
    _nc_cache['nc'] = nc
    return nc


# ---------------- host-side preparation ----------------

def _voxel_host(pts):
    # identical f32 ops to the reference
    x = pts[:, 0].astype(np.float32)
    y = pts[:, 1].astype(np.float32)
    ux = (x - X_MIN) / VX
    uy = (y - X_MIN) / VX
    ix = np.clip(np.floor(ux), 0, 511).astype(np.int32)
    iy = np.clip(np.floor(uy), 0, 511).astype(np.int32)
    return ix, iy


def _prep_unit(pts, strip, vox):
    """pts: [N, 3] points of one cloud. Returns per-unit device inputs for
    points whose global ix falls in this strip: pts [128, MC*4] f32
    (xr, yr, z, w in [partition, column] token layout), aux [128, MC*4] f32
    (center-x, center-y, quarter-id, 0), idxv [128, MC*8] i16 (wrapped
    scatter idx, replicated per Q7 core), idxq [128, MC*4] i16 (per-quarter
    clamped gather idx, per-core wrap)."""
    ix, iy = vox
    m = (ix >> 6) == strip
    p = pts[m].astype(np.float32)
    ixs = (ix[m] - 64 * strip).astype(np.int64)
    iys = iy[m].astype(np.int64)
    n = p.shape[0]
    assert n <= M, (n, M)

    xsh = np.float32(X_MIN + strip * np.float64(64) * np.float64(VX))
    xr = (p[:, 0] - xsh).astype(np.float32)
    yr = (p[:, 1] - X_MIN).astype(np.float32)
    z = p[:, 2].astype(np.float32)

    vid = ixs * 512 + iys
    # rank within pillar
    order = np.argsort(vid, kind="stable")
    sv = vid[order]
    first = np.ones(n, bool)
    first[1:] = sv[1:] != sv[:-1]
    seg_start = np.maximum.accumulate(np.where(first, np.arange(n), 0))
    rank_sorted = np.arange(n) - seg_start
    rank = np.empty(n, np.int64)
    rank[order] = rank_sorted

    # empty pillar for padding
    used = np.zeros(SP, bool)
    used[vid] = True
    E = int(np.argmin(used))
    assert not used[E]
    ex = np.float32((E // 512 + 0.5) * 0.2)
    ey = np.float32((E % 512 + 0.5) * 0.2)

    pts4 = np.empty((M, 4), np.float32)
    pts4[:, 0] = ex
    pts4[:, 1] = ey
    pts4[:, 2] = 0.0
    pts4[:, 3] = 0.0
    aux4 = np.zeros((M, 4), np.float32)
    vidA = np.full(M, E, np.int64)
    off = 0
    for r, cap in enumerate(R_CAPS):
        s = np.where(rank == r)[0] if r < N_ROUNDS - 1 else np.where(rank >= r)[0]
        if r == N_ROUNDS - 1:
            assert (rank[s] == r).all(), "rank overflow beyond round capacity"
        assert len(s) <= cap * 128, (r, len(s))
        k = len(s)
        pts4[off:off + k, 0] = xr[s]
        pts4[off:off + k, 1] = yr[s]
        pts4[off:off + k, 2] = z[s]
        pts4[off:off + k, 3] = 1.0
        aux4[off:off + k, 0] = ((ixs[s] + 0.5) * np.float64(VX)).astype(np.float32)
        aux4[off:off + k, 1] = ((iys[s] + 0.5) * np.float64(VX)).astype(np.float32)
        vidA[off:off + k] = vid[s]
        off += cap * 128
    aux4[:, 2] = (vidA >> 13).astype(np.float32)

    ptsH = pts4.reshape(MC, 128, 4).transpose(1, 0, 2).reshape(128, MC * 4)
    auxH = aux4.reshape(MC, 128, 4).transpose(1, 0, 2).reshape(128, MC * 4)
    v16 = vidA.astype(np.int16)
    idxvH = np.tile(v16.reshape(MC * 8, 16).T, (8, 1))
    idxqH = np.empty((128, MC * 4), np.int16)
    for q in range(4):
        vq = np.clip(vidA - 8192 * q, 0, 8191).astype(np.int16)
        idxqH[:, q * MC:(q + 1) * MC] = (
            vq.reshape(8, MC, 16).transpose(0, 2, 1).reshape(128, MC))
    return {"pts": np.ascontiguousarray(ptsH),
            "aux": np.ascontiguousarray(auxH),
            "idxv": np.ascontiguousarray(idxvH),
            "idxq": np.ascontiguousarray(idxqH)}


def kernel(pc0s, pc1s, W, bn_gamma, bn_beta, bn_mean, bn_var):
    from concourse.bass_utils import run_bass_kernel_spmd

    pc0s = np.asarray(pc0s, np.float32)
    pc1s = np.asarray(pc1s, np.float32)
    W = np.asarray(W, np.float32)
    scale = (np.asarray(bn_gamma, np.float32)
             / np.sqrt(np.asarray(bn_var, np.float32) + np.float32(1e-3)))
    W_eff = W * scale[None, :]
    bias = np.asarray(bn_beta, np.float32) - np.asarray(bn_mean, np.float32) * scale
    W0, W1, W2 = W_eff[0:3], W_eff[3:6], W_eff[6:9]

    def wc_for(strip):
        Sx = np.float32(X_MIN + strip * 12.8)
        Sy = X_MIN
        wc = np.zeros((9, 64), np.float32)
        wc[0] = W0[0]
        wc[1] = W0[1]
        wc[2] = W0[2] + W2[2]
        wc[3:6] = W1
        wc[6] = W2[0]
        wc[7] = W2[1]
        wc[8] = bias + Sx * W0[0] + Sy * W0[1]
        return wc

    clouds = [pc0s, pc1s]
    vox = {(c, b): _voxel_host(clouds[c][b]) for c in range(2) for b in range(2)}
    wcs = [wc_for(s) for s in range(N_STRIPS)]
    in_maps = []
    for core in range(8):
        b, sp = core // 4, core % 4
        ins = {}
        for u in range(4):
            strip = 2 * sp + u // 2
            cloud = u % 2
            prep = _prep_unit(clouds[cloud][b], strip, vox[(cloud, b)])
            ins[f"pts{u}"] = prep["pts"]
            ins[f"aux{u}"] = prep["aux"]
            ins[f"idxv{u}"] = prep["idxv"]
            ins[f"idxq{u}"] = prep["idxq"]
            ins[f"wc{u}"] = wcs[strip]
        in_maps.append(ins)

    nc = _build_nc()
    import time as _time
    _t0 = _time.time()
    res = run_bass_kernel_spmd(nc, in_maps, list(range(8)))
    kernel.last_device_wall = _time.time() - _t0

    out = np.zeros((2, GX * GX, 64), np.float32)
    for core in range(8):
        b, sp = core // 4, core % 4
        o = res.results[core]["out"]        # [2*SP, 64]
        for sl in range(2):
            strip = 2 * sp + sl
            out[b, strip * SP:(strip + 1) * SP] = o[sl * SP:(sl + 1) * SP]
    return out.reshape(2, GX, GX, 64)


if __name__ == "__main__":
    # smoke test against reference (reference on CPU backend)
    sys.path.insert(0, '/root/problem')
    import jax
    import reference
    cpu = jax.devices("cpu")[0]
    with jax.default_device(cpu):
        inputs = {k: np.asarray(v) for k, v in reference.setup_inputs().items()}
        exp = np.asarray(reference.reference(**inputs))
    got = kernel(**inputs)
    err = np.abs(got - exp)
    den = np.abs(exp).max()
    print("max abs err:", err.max(), " absmax:", den, " rel:", err.max() / den)


# revision 35
# speedup vs baseline: 3.0947x; 2.0241x over previous
"""Trainium2 Bass kernel for nn_AccFlow2FrameEncoder (PointPillars-style encoder).

Math per (batch, cloud):
  voxelize points into 512x512 grid; cluster-mean of xyz per pillar; features
  f = [pts, pts-cm, pts-center] (9); h = relu(BN(f @ W)); scatter-mean h per
  pillar. Output = grid(pc1) - grid(pc0), shape [2, 512, 512, 64] f32.

Sharding: 8 cores = 2 batches x 4 strip-pairs. Each core handles 2 strips of
64 ix-rows (32768 pillars each) x both clouds, so the diff is formed on-core.
Host partitions points by strip, orders them into collision-free "rounds"
(rank within pillar) so every on-device dma_scatter_add call has unique
pillar indices (exact f32 accumulation; duplicate indices in one call race).
Host also precomputes the wrapped i16 index lists (scatter + per-quarter
gather) and per-token aux data (pillar-center coords, quarter id), loaded as
contiguous DMA inputs -- no on-device index arithmetic or DRAM round trips.

Device pipeline per unit (strip x cloud):
  1. dma_scatter_add [x,y,z,w] into SBUF parity tables (unique-idx rounds)
  2. PE-transpose the parity tables, DMA to a component-major DRAM table
     [4, 32768] with 512B-contiguous descriptors
  3. per quarter-strip: load a [128, 8192] replicated SBUF table and ap_gather
     per-point pillar sums (per-Q7-core idx lists); merge quarters by mask
  4. build 9-feature vectors (cluster-mean + center + ones-bias features);
     PE transpose + matmul (9->64); ACT relu scaled by validity w
  5. dma_scatter_add [w, h] (65 f32) into SBUF parity tables (unique rounds)
  6. after both clouds: normalize by counts, subtract, DMA to output grid

(dma_gather/InstDMAGatherAnt is avoided deliberately: it crashes at runtime
under the axon/PJRT path; SBUF-only custom ops work.)
"""
import sys
sys.path.insert(0, '/opt/trn_rl_repo')

import numpy as np

VX = np.float32(0.2)
X_MIN = np.float32(-51.2)
GX = 512
N_STRIPS = 8          # strips of 64 ix rows
SP = 32768            # pillars per strip
R_CAPS = [90, 20, 4, 2, 1, 1, 1, 1]   # per-round capacity in 128-token units
MC = sum(R_CAPS)      # 120 -> M = 15360 tokens per unit
M = MC * 128
N_ROUNDS = len(R_CAPS)


def _chunks(max_cap=24):
    """(offset, size) sub-chunks of the rounds, each <= max_cap 128-token
    units, never crossing a round boundary (keeps per-call indices unique)."""
    out = []
    off = 0
    for cap in R_CAPS:
        o = 0
        while o < cap:
            c = min(max_cap, cap - o)
            out.append((off + o, c))
            o += c
        off += cap
    return out

_nc_cache = {}


def _build_nc():
    if 'nc' in _nc_cache:
        return _nc_cache['nc']
    import concourse.bass as bass
    import concourse.tile as tile
    from concourse import bacc, mybir
    from concourse.bass import AP
    from concourse.masks import make_identity
    from concourse.library_config import mlp, ap_gather as apg_lib

    F32 = mybir.dt.float32
    I16 = mybir.dt.int16
    I8 = mybir.dt.int8
    Alu = mybir.AluOpType
    Act = mybir.ActivationFunctionType

    # raise stale SBUF cap if present
    try:
        import concourse.tile_utils as tile_utils
        if getattr(tile_utils, 'max_sbuf_usage', None):
            tile_utils.max_sbuf_usage = 206 * 1024
    except Exception:
        pass

    nc = bacc.Bacc(None, target_bir_lowering=False, debug=False)

    pts_d = [nc.dram_tensor(f"pts{u}", [128, MC * 4], F32, kind="ExternalInput") for u in range(4)]
    aux_d = [nc.dram_tensor(f"aux{u}", [128, MC * 4], F32, kind="ExternalInput") for u in range(4)]
    idxv_d = [nc.dram_tensor(f"idxv{u}", [128, MC * 8], I16, kind="ExternalInput") for u in range(4)]
    idxw_d = [nc.dram_tensor(f"idxw{u}", [128, MC * 8], I16, kind="ExternalInput") for u in range(4)]
    idxq_d = [nc.dram_tensor(f"idxq{u}", [128, MC], I16, kind="ExternalInput") for u in range(4)]
    wc_d = [nc.dram_tensor(f"wc{u}", [96, 64], F32, kind="ExternalInput") for u in range(4)]
    t1t_d = [nc.dram_tensor(f"t1t{u}", [4, SP], F32, kind="Internal") for u in range(2)]
    out_d = nc.dram_tensor("out", [2 * SP, 64], F32, kind="ExternalOutput")

    with tile.TileContext(nc) as tc:
        with tc.tile_pool(name="sb", bufs=1) as pool, \
             tc.tile_pool(name="sb2", bufs=2) as pool2, \
             tc.tile_pool(name="ps", bufs=2, space="PSUM") as psum, \
             tc.tile_pool(name="psg", bufs=1, space="PSUM") as psumg, \
             tc.tile_pool(name="psq", bufs=3, space="PSUM") as psumq:

            nc.gpsimd.load_library(mlp)
            ident = pool.tile([128, 128], F32, tag="ident")
            make_identity(nc, ident[:])

            import os as _os
            _skip = set(_os.environ.get("KERNEL_SKIP", "").split(","))
            _reps = int(_os.environ.get("KERNEL_REPS", "1"))
            # feature tensor fc [128, MC, 32]: cols 0:8 rewritten per unit,
            # col 8 constant ones (bias), cols 9:32 constant zero pad; 32-wide
            # so 3 chunks transpose together with matmul base partitions 0/32/64
            fc = pool.tile([128, MC, 32], F32, tag="fc")
            nc.vector.memset(fc[:], 0.0)
            nc.vector.memset(fc[:, :, 8], 1.0)
            for _rep in range(_reps):
              for u in range(4):
                strip_l, cloud = u // 2, u % 2

                pts_t = pool2.tile([128, MC, 4], F32, tag="pts")
                nc.sync.dma_start(pts_t[:], pts_d[u].ap())
                aux_t = pool2.tile([128, MC, 4], F32, tag="aux")
                nc.sync.dma_start(aux_t[:], aux_d[u].ap())
                idxv = pool2.tile([128, MC * 8], I16, tag="idxv")
                nc.sync.dma_start(idxv[:], idxv_d[u].ap())
                idxw = pool2.tile([128, MC * 8], I16, tag="idxw")
                nc.sync.dma_start(idxw[:], idxw_d[u].ap())
                idxQs = pool2.tile([128, MC], I16, tag="idxq")
                nc.sync.dma_start(idxQs[:], idxq_d[u].ap())
                wc_t = pool2.tile([96, 64], F32, tag="wc")
                nc.sync.dma_start(wc_t[:], wc_d[u].ap())

                def T(tag):
                    return pool.tile([128, MC], F32, tag=tag, name=f"{tag}_u{u}")
                tmp = T("tmp")

                # ---- T1 scatter rounds (xyzw, elem 4)
                t1A = pool.tile([128, 128, 4], F32, tag="t1A")
                t1B = pool.tile([128, 128, 4], F32, tag="t1B")
                nc.vector.memset(t1A[:], 0.0)
                nc.vector.memset(t1B[:], 0.0)
                for off, cap in (() if "t1" in _skip else _chunks()):
                    n = cap * 128
                    nc.gpsimd.dma_scatter_add(
                        t1A[:], pts_t[:, off:off + cap, :],
                        idxv[:, off * 8:(off + cap) * 8],
                        n, n, 4,
                        sbuf_tokens_per_rank=128, parity_reg=0, out_ap_other=t1B[:])

                # ---- export component-major table [4, 32768] via PE transpose
                # (512B-contiguous descriptors instead of 4B)
                t1t = t1t_d[cloud]
                for par, tbl in ((0, t1A), (1, t1B)):
                    ptT = psumg.tile([128, 4, 128], F32, tag="t1T")
                    for r in range(4):
                        nc.tensor.transpose(
                            out=ptT[:, r, :], in_=tbl[:, :, r], identity=ident[:])
                    t1sb = pool.tile([128, 4, 128], F32, tag="t1Tsb")
                    nc.vector.tensor_copy(t1sb[:], ptT[:])
                    dst = bass.AP(t1t, 128 * par,
                                  [[256, 128], [32768, 4], [1, 128]])
                    nc.sync.dma_start(dst, t1sb[:])

                # ---- ap_gather per quarter-strip: each Q7 core's 16 table
                # rows hold comp r of pillar sub-block s (row 16c+4s+r,
                # 2048 pillars each), so the gather table free size is 2048.
                # One shared idx list (vid & 2047). Gather output is per-core
                # (token i of core c at row 16c+4s+r, col i): PE-transpose
                # [128,128] col-blocks into per-token layout in PSUM, then
                # region-select ((vid>>11) == 4q+s) straight into sel.
                nc.gpsimd.load_library(apg_lib)
                sel = pool.tile([128, MC, 4], F32, tag="sel")
                tmp4 = pool.tile([128, MC, 4], F32, tag="tmp4")
                mbs = [pool.tile([128, MC], I8, tag=f"mb{s}",
                                 name=f"mb{s}_u{u}") for s in range(4)]
                if _os.environ.get("KERNEL_SIM") or "cm" in _skip:
                    nc.vector.memset(sel[:], 0.0)
                sel_r = sel[:].rearrange("p (c w) r -> p w c r", c=8)
                for q in (() if "cm" in _skip else range(4)):
                    tab = pool2.tile([128, 2048], F32, tag="tab",
                                     name=f"tab{u}_{q}")
                    srcap = bass.AP(t1t, q * 8192,
                                    [[2048, 4], [32768, 4], [1, 2048]])
                    for c in range(8):
                        nc.sync.dma_start(tab[16 * c:16 * c + 16, :], srcap)
                    gb = pool2.tile([128, MC * 16], F32, tag="gout",
                                    name=f"gout{u}_{q}")
                    if "gath" not in _skip:
                        nc.gpsimd.ap_gather(gb[:], tab[:], idxQs[:],
                                            128, 2048, 1, MC * 16)
                    for w0, wn in ((0, 8), (8, 7)):
                        pq = psumg.tile([128, 8, 128], F32, tag="pq")
                        for w in range(wn):
                            nc.tensor.transpose(
                                out=pq[:, w, :],
                                in_=gb[:, 128 * (w0 + w):128 * (w0 + w) + 128],
                                identity=ident[:])
                        pq_r = pq[:].rearrange("p w (c s r) -> p s w c r",
                                               c=8, s=4)
                        for s in range(4):
                            if w0 == 0:
                                nc.vector.tensor_scalar(
                                    mbs[s][:], aux_t[:, :, 2],
                                    float(4 * q + s), None, op0=Alu.is_equal)
                            mb_r = mbs[s][:].rearrange("p (c w) -> p w c", c=8)
                            nc.vector.copy_predicated(
                                sel_r[:, w0:w0 + wn],
                                mb_r[:, w0:w0 + wn].to_broadcast(
                                    [128, wn, 8, 4]),
                                pq_r[:, s, 0:wn])
                nc.gpsimd.load_library(mlp)

                # rn = w / max(pillar_count, 1): per-token scatter-mean weight
                rw = T("rw")
                rn = T("rn")
                nc.vector.tensor_scalar_max(tmp[:], sel[:, :, 3], 1.0)
                nc.vector.reciprocal(rw[:], tmp[:])
                nc.vector.tensor_tensor(out=rn[:], in0=pts_t[:, :, 3],
                                        in1=rw[:], op=Alu.mult)

                # ---- feature tensor (fc allocated/zero-padded outside loop)
                nc.vector.tensor_copy(fc[:, :, 0:3], pts_t[:, :, 0:3])
                tmp3 = tmp4[:, :, 0:3]
                nc.vector.tensor_tensor(out=tmp3, in0=sel[:, :, 0:3],
                                        in1=rw[:].to_broadcast([128, MC, 3]), op=Alu.mult)
                nc.vector.tensor_tensor(out=fc[:, :, 3:6], in0=pts_t[:, :, 0:3],
                                        in1=tmp3, op=Alu.subtract)
                nc.vector.tensor_tensor(out=fc[:, :, 6], in0=pts_t[:, :, 0],
                                        in1=aux_t[:, :, 0], op=Alu.subtract)
                nc.vector.tensor_tensor(out=fc[:, :, 7], in0=pts_t[:, :, 1],
                                        in1=aux_t[:, :, 1], op=Alu.subtract)

                # ---- PE: transpose 3 chunks at a time (feature rows at base
                # partitions 0/32/64), matmul 9->64, ACT relu scaled by w/n
                h_t = pool.tile([128, MC, 64], F32, tag="ht")
                for c3 in (() if "pe" in _skip else range(MC // 3)):
                    pt = psum.tile([96, 128], F32, tag="pt")
                    nc.tensor.transpose(
                        out=pt[:],
                        in_=fc[:, 3 * c3:3 * c3 + 3, :],
                        identity=ident[:])
                    fT = pool2.tile([96, 128], F32, tag="fT")
                    nc.vector.tensor_copy(fT[:], pt[:])
                    for k in range(3):
                        ch = 3 * c3 + k
                        qp = psumq.tile([128, 64], F32, tag="q")
                        nc.tensor.matmul(out=qp[:], lhsT=fT[32 * k:32 * k + 9, :],
                                         rhs=wc_t[32 * k:32 * k + 9, :],
                                         start=True, stop=True)
                        nc.scalar.activation(h_t[:, ch, :], qp[:], Act.Relu,
                                             scale=rn[:, ch:ch + 1])
                if cloud == 0:
                    # diff = grid1 - grid0: negate cloud-0 h before accumulate
                    nc.scalar.activation(h_t[:], h_t[:], Act.Copy, scale=-1.0)

                # ---- T2 scatter rounds (h/n, elem 64) into the shared
                # per-strip pair; remapped idx puts pillar 2(p+128g) at A[p,g]
                # and 2(p+128g)+1 at B[p,g] so both clouds accumulate the diff
                t2A = pool.tile([128, 128, 64], F32, tag="t2A")
                t2B = pool.tile([128, 128, 64], F32, tag="t2B")
                if cloud == 0:
                    nc.vector.memset(t2A[:], 0.0)
                    nc.vector.memset(t2B[:], 0.0)
                for off, cap in (() if "t2" in _skip else _chunks()):
                    n = cap * 128
                    nc.gpsimd.dma_scatter_add(
                        t2A[:], h_t[:, off:off + cap, :],
                        idxw[:, off * 8:(off + cap) * 8],
                        n, n, 64,
                        sbuf_tokens_per_rank=128, parity_reg=0, out_ap_other=t2B[:])

                # ---- export after cloud 1: tables already hold the diff
                if cloud == 1:
                    for par, tbl in ((0, t2A), (1, t2B)):
                        dst = bass.AP(out_d, strip_l * SP * 64 + 64 * par,
                                      [[128, 128], [16384, 128], [1, 64]])
                        nc.sync.dma_start(dst, tbl[:])

    nc.compile()
    _nc_cache['nc'] = nc
    return nc


# ---------------- host-side preparation ----------------

def _voxel_host(pts):
    # identical f32 ops to the reference
    x = pts[:, 0].astype(np.float32)
    y = pts[:, 1].astype(np.float32)
    ux = (x - X_MIN) / VX
    uy = (y - X_MIN) / VX
    ix = np.clip(np.floor(ux), 0, 511).astype(np.int32)
    iy = np.clip(np.floor(uy), 0, 511).astype(np.int32)
    return ix, iy


def _prep_unit(pts, strip, vox):
    """pts: [N, 3] points of one cloud. Returns per-unit device inputs for
    points whose global ix falls in this strip: pts [128, MC*4] f32
    (xr, yr, z, w in [partition, column] token layout), aux [128, MC*4] f32
    (center-x, center-y, quarter-id, 0), idxv [128, MC*8] i16 (wrapped
    scatter idx, replicated per Q7 core), idxq [128, MC*4] i16 (per-quarter
    clamped gather idx, per-core wrap)."""
    ix, iy = vox
    m = (ix >> 6) == strip
    p = pts[m].astype(np.float32)
    ixs = (ix[m] - 64 * strip).astype(np.int64)
    iys = iy[m].astype(np.int64)
    n = p.shape[0]
    assert n <= M, (n, M)

    xsh = np.float32(X_MIN + strip * np.float64(64) * np.float64(VX))
    xr = (p[:, 0] - xsh).astype(np.float32)
    yr = (p[:, 1] - X_MIN).astype(np.float32)
    z = p[:, 2].astype(np.float32)

    vid = ixs * 512 + iys
    # rank within pillar
    order = np.argsort(vid, kind="stable")
    sv = vid[order]
    first = np.ones(n, bool)
    first[1:] = sv[1:] != sv[:-1]
    seg_start = np.maximum.accumulate(np.where(first, np.arange(n), 0))
    rank_sorted = np.arange(n) - seg_start
    rank = np.empty(n, np.int64)
    rank[order] = rank_sorted

    # empty pillar for padding
    used = np.zeros(SP, bool)
    used[vid] = True
    E = int(np.argmin(used))
    assert not used[E]
    ex = np.float32((E // 512 + 0.5) * 0.2)
    ey = np.float32((E % 512 + 0.5) * 0.2)

    pts4 = np.empty((M, 4), np.float32)
    pts4[:, 0] = ex
    pts4[:, 1] = ey
    pts4[:, 2] = 0.0
    pts4[:, 3] = 0.0
    aux4 = np.zeros((M, 4), np.float32)
    vidA = np.full(M, E, np.int64)
    off = 0
    for r, cap in enumerate(R_CAPS):
        s = np.where(rank == r)[0] if r < N_ROUNDS - 1 else np.where(rank >= r)[0]
        if r == N_ROUNDS - 1:
            assert (rank[s] == r).all(), "rank overflow beyond round capacity"
        assert len(s) <= cap * 128, (r, len(s))
        k = len(s)
        pts4[off:off + k, 0] = xr[s]
        pts4[off:off + k, 1] = yr[s]
        pts4[off:off + k, 2] = z[s]
        pts4[off:off + k, 3] = 1.0
        aux4[off:off + k, 0] = ((ixs[s] + 0.5) * np.float64(VX)).astype(np.float32)
        aux4[off:off + k, 1] = ((iys[s] + 0.5) * np.float64(VX)).astype(np.float32)
        vidA[off:off + k] = vid[s]
        off += cap * 128
    aux4[:, 2] = (vidA >> 11).astype(np.float32)  # region id (2048-pillar blk)

    ptsH = pts4.reshape(MC, 128, 4).transpose(1, 0, 2).reshape(128, MC * 4)
    auxH = aux4.reshape(MC, 128, 4).transpose(1, 0, 2).reshape(128, MC * 4)
    v16 = vidA.astype(np.int16)
    idxvH = np.tile(v16.reshape(MC * 8, 16).T, (8, 1))
    # T2 remap: pillar v -> u with partition (v>>1)&127, slot v>>8, parity v&1
    # => A[p,g] = pillar 2(p+128g), B[p,g] = 2(p+128g)+1 (export-friendly)
    uA = (((vidA >> 1) & 127) + 128 * (2 * (vidA >> 8) + (vidA & 1))).astype(np.int16)
    idxwH = np.tile(uA.reshape(MC * 8, 16).T, (8, 1))
    vg = (vidA & 2047).astype(np.int16)   # within-2048-block gather idx
    idxqH = vg.reshape(8, MC, 16).transpose(0, 2, 1).reshape(128, MC)
    return {"pts": np.ascontiguousarray(ptsH),
            "aux": np.ascontiguousarray(auxH),
            "idxv": np.ascontiguousarray(idxvH),
            "idxw": np.ascontiguousarray(idxwH),
            "idxq": np.ascontiguousarray(idxqH)}


def kernel(pc0s, pc1s, W, bn_gamma, bn_beta, bn_mean, bn_var):
    from concourse.bass_utils import run_bass_kernel_spmd

    pc0s = np.asarray(pc0s, np.float32)
    pc1s = np.asarray(pc1s, np.float32)
    W = np.asarray(W, np.float32)
    scale = (np.asarray(bn_gamma, np.float32)
             / np.sqrt(np.asarray(bn_var, np.float32) + np.float32(1e-3)))
    W_eff = W * scale[None, :]
    bias = np.asarray(bn_beta, np.float32) - np.asarray(bn_mean, np.float32) * scale
    W0, W1, W2 = W_eff[0:3], W_eff[3:6], W_eff[6:9]

    def wc_for(strip):
        Sx = np.float32(X_MIN + strip * 12.8)
        Sy = X_MIN
        wc = np.zeros((9, 64), np.float32)
        wc[0] = W0[0]
        wc[1] = W0[1]
        wc[2] = W0[2] + W2[2]
        wc[3:6] = W1
        wc[6] = W2[0]
        wc[7] = W2[1]
        wc[8] = bias + Sx * W0[0] + Sy * W0[1]
        # replicated at base partitions 0/32/64 to pair with 3-chunk lhsT
        wc3 = np.zeros((96, 64), np.float32)
        for k in range(3):
            wc3[32 * k:32 * k + 9] = wc
        return wc3

    clouds = [pc0s, pc1s]
    vox = {(c, b): _voxel_host(clouds[c][b]) for c in range(2) for b in range(2)}
    wcs = [wc_for(s) for s in range(N_STRIPS)]
    in_maps = []
    for core in range(8):
        b, sp = core // 4, core % 4
        ins = {}
        for u in range(4):
            strip = 2 * sp + u // 2
            cloud = u % 2
            prep = _prep_unit(clouds[cloud][b], strip, vox[(cloud, b)])
            ins[f"pts{u}"] = prep["pts"]
            ins[f"aux{u}"] = prep["aux"]
            ins[f"idxv{u}"] = prep["idxv"]
            ins[f"idxw{u}"] = prep["idxw"]
            ins[f"idxq{u}"] = prep["idxq"]
            ins[f"wc{u}"] = wcs[strip]
        in_maps.append(ins)

    nc = _build_nc()
    import time as _time
    _t0 = _time.time()
    res = run_bass_kernel_spmd(nc, in_maps, list(range(8)))
    kernel.last_device_wall = _time.time() - _t0

    out = np.zeros((2, GX * GX, 64), np.float32)
    for core in range(8):
        b, sp = core // 4, core % 4
        o = res.results[core]["out"]        # [2*SP, 64]
        for sl in range(2):
            strip = 2 * sp + sl
            out[b, strip * SP:(strip + 1) * SP] = o[sl * SP:(sl + 1) * SP]
    return out.reshape(2, GX, GX, 64)


if __name__ == "__main__":
    # smoke test against reference (reference on CPU backend)
    sys.path.insert(0, '/root/problem')
    import jax
    import reference
    cpu = jax.devices("cpu")[0]
    with jax.default_device(cpu):
        inputs = {k: np.asarray(v) for k, v in reference.setup_inputs().items()}
        exp = np.asarray(reference.reference(**inputs))
    got = kernel(**inputs)
    err = np.abs(got - exp)
    den = np.abs(exp).max()
    print("max abs err:", err.max(), " absmax:", den, " rel:", err.max() / den)
